# revision 5
# baseline (speedup 1.0000x reference)
"""Trainium2 Bass kernel for nn_AttentionPoolingLayer.

Math (per batch b, per t):
  att_in = [qt, k, qt-k, qt*k]  (4*64 features)
  h1 = prelu(att_in @ W1, a1); h2 = prelu(h1 @ W2, a2); h3 = prelu(h2 @ W3, a3)
  score = h3 @ Wl ; out[b] = sum_t score[b,t] * k[b,t]

Device mapping: pure data-parallel over batch (8 cores x 256 batches).
Weight folding: att_in @ W1 = qt@(W1a+W1c) + k@(W1b-W1c) + (qt*k)@W1d,
so only features [qt; k; qt*k] are materialized (feature-major / transposed
activations throughout; PReLU is fused into the PSUM->SBUF eviction on the
scalar engine via the Prelu activation function with scalar alpha).
"""
import sys

sys.path.insert(0, "/opt/trn_rl_repo")

import numpy as np
import ml_dtypes

# problem shapes (hardcoded per spec)
B, T, D = 2048, 200, 64
H1, H2, H3 = 256, 128, 64
M = 8            # cores
BC = B // M      # 256 batches per core
NT = BC // 2     # 128 tiles of 2 batches (400 rows) per core
TA, TB = 128, 72  # t-chunks (200 = 128 + 72)
GRP = 8          # batches per DMA load group
NG = BC // GRP   # 32 load groups
BNC = 8          # bounce chunks for the fp32->bf16 cast


_CACHE = {}


def _build():
    import concourse.bass as bass
    import concourse.bacc as bacc
    import concourse.tile as tile
    from concourse import mybir
    from concourse.masks import make_identity

    bf16 = mybir.dt.bfloat16
    f32 = mybir.dt.float32
    AF = mybir.ActivationFunctionType
    ALU = mybir.AluOpType

    nc = bacc.Bacc(None, target_bir_lowering=False, debug=True)

    k_in = nc.declare_dram_parameter("k", [BC, T, D], f32, isOutput=False)
    q_in = nc.declare_dram_parameter("q", [BC, D], f32, isOutput=False)
    wqk_in = nc.declare_dram_parameter("wqk", [128, H1], f32, isOutput=False)
    wm_in = nc.declare_dram_parameter("wm", [64, H1], f32, isOutput=False)
    w2_in = nc.declare_dram_parameter("w2", [2, 128, H2], f32, isOutput=False)
    w3_in = nc.declare_dram_parameter("w3", [H2, H3], f32, isOutput=False)
    wlr_in = nc.declare_dram_parameter("wlr", [H3, 64], f32, isOutput=False)
    alphas_in = nc.declare_dram_parameter("alphas", [1, 4], f32, isOutput=False)
    out_t = nc.declare_dram_parameter("out_t", [D, BC], f32, isOutput=True)

    # bf16 bounce of k in DRAM (written by SWDGE cast DMA, read by HWDGE)
    k_bf = nc.dram_tensor("k_bf", [BC, T, D], bf16)

    with tile.TileContext(nc) as tc:
        with (
            tc.tile_pool(name="fix", bufs=1) as fix,
            tc.tile_pool(name="ld", bufs=2) as ld,
            tc.tile_pool(name="act", bufs=2) as actp,
            tc.tile_pool(name="rhsp", bufs=3) as rhsp,
            tc.tile_pool(name="ps_kt", bufs=1, space="PSUM") as pskt,
            tc.tile_pool(name="ps_h1", bufs=2, space="PSUM") as psh1,
            tc.tile_pool(name="ps_h2", bufs=2, space="PSUM") as psh2,
            tc.tile_pool(name="ps_mix", bufs=1, space="PSUM") as psmix,
        ):
            ident = fix.tile([128, 128], bf16)
            make_identity(nc, ident)

            # --- weights (bf16 on chip; cast during SWDGE DMA)
            wqk = fix.tile([128, H1], bf16)
            nc.gpsimd.dma_start(out=wqk[:, :], in_=wqk_in[:, :])
            wm = fix.tile([128, H1], bf16)
            nc.gpsimd.dma_start(out=wm[64:128, :], in_=wm_in[:, :])
            w2 = fix.tile([128, 2, H2], bf16)
            nc.gpsimd.dma_start(out=w2[:, :, :], in_=w2_in[:, :, :].rearrange("c k h -> k c h"))
            w3 = fix.tile([H2, H3], bf16)
            nc.gpsimd.dma_start(out=w3[:, :], in_=w3_in[:, :])
            wlr = fix.tile([H3, 64], bf16)
            nc.gpsimd.dma_start(out=wlr[:, :], in_=wlr_in[:, :])
            # alpha values replicated across all 128 partitions (DMA bcast)
            alphas = fix.tile([128, 4], f32)
            alphas_bcast = bass.AP(
                tensor=alphas_in.ap().tensor, offset=0, ap=[[0, 128], [1, 4]]
            )
            nc.gpsimd.dma_start(out=alphas[:, :], in_=alphas_bcast)

            # --- qT [64, BC] bf16: cast-load q then PE-transpose
            qsb = fix.tile([128, 2, D], bf16)
            nc.gpsimd.dma_start(
                out=qsb[:, :, :], in_=q_in[:, :].rearrange("(g p) d -> p g d", p=128)
            )
            ps_q = psh1.tile([64, BC], bf16, tag="h1")
            for g in range(2):
                nc.tensor.transpose(ps_q[:, g * 128 : (g + 1) * 128], qsb[:, g, :], ident[:, :])
            qT = fix.tile([64, BC], bf16)
            nc.vector.tensor_copy(qT[:, :], ps_q[:, :])

            # --- k fp32 -> bf16 bounce in DRAM (chunked so loads can pipeline)
            for c in range(BNC):
                s = c * (BC // BNC)
                e = s + BC // BNC
                nc.gpsimd.dma_start(out=k_bf[s:e, :, :], in_=k_in[s:e, :, :])

            # --- persistent output accumulator [x, 2, 200] layouts
            outT = fix.tile([128, BC], f32)

            prelu_kw = dict(func=AF.Prelu, bias=0.0, scale=1.0)

            for g in range(NG):
                b0 = g * GRP
                # group load: kA [128, (8, 64)], kB [72, (8, 64)] bf16 (HWDGE, no cast)
                kA = ld.tile([TA, GRP, D], bf16, tag="kA")
                nc.sync.dma_start(
                    out=kA[:, :, :],
                    in_=k_bf[b0 : b0 + GRP, 0:TA, :].rearrange("b t d -> t b d"),
                )
                kB = ld.tile([TB, GRP, D], bf16, tag="kB")
                nc.sync.dma_start(
                    out=kB[:, :, :],
                    in_=k_bf[b0 : b0 + GRP, TA:T, :].rearrange("b t d -> t b d"),
                )

                for j in range(GRP // 2):
                    i = g * (GRP // 2) + j    # tile index
                    bb = 2 * j               # batch within group

                    # 1) transposes -> ps_kt upper partitions
                    ps_kt = pskt.tile([128, 400], bf16, tag="kt")
                    for b in range(2):
                        nc.tensor.transpose(
                            ps_kt[64:128, b * T : b * T + TA], kA[:, bb + b, :], ident[:, :]
                        )
                        nc.tensor.transpose(
                            ps_kt[64:128, b * T + TA : (b + 1) * T],
                            kB[:, bb + b, :],
                            ident[0:TB, 0:TB],
                        )

                    # 2) rhs tile: [qb(0:64); kt(64:128)]
                    rhs = rhsp.tile([128, 400], bf16, tag="rhs")
                    nc.vector.tensor_copy(rhs[64:128, :], ps_kt[64:128, :])
                    qsrc = qT[:, 2 * i : 2 * i + 2]
                    qsrc_b = bass.AP(
                        tensor=qsrc.tensor, offset=qsrc.offset,
                        ap=[qsrc.ap[0], [1, 2], [0, T]],
                    )
                    nc.gpsimd.tensor_copy(
                        out=rhs[0:64, :].rearrange("p (b t) -> p b t", b=2), in_=qsrc_b
                    )
                    # qb copy #2 on upper partitions + mT = kt * qb
                    scr = rhsp.tile([128, 400], bf16, tag="scr")
                    nc.gpsimd.tensor_copy(
                        out=scr[64:128, :].rearrange("p (b t) -> p b t", b=2), in_=qsrc_b
                    )
                    mt = rhsp.tile([128, 400], bf16, tag="mt")
                    nc.gpsimd.tensor_tensor(
                        out=mt[64:128, :], in0=rhs[64:128, :], in1=scr[64:128, :],
                        op=ALU.mult,
                    )

                    # 3) GEMM1 -> ps_h1 (2 banks, chunks at col 0 and 512)
                    ps_h1 = psh1.tile([128, 1024], mybir.dt.float32, tag="h1")
                    for c in range(2):
                        nc.tensor.matmul(
                            ps_h1[:, c * 512 : c * 512 + 400],
                            wqk[:, c * 128 : (c + 1) * 128],
                            rhs[:, :],
                            start=True, stop=False,
                        )
                        nc.tensor.matmul(
                            ps_h1[:, c * 512 : c * 512 + 400],
                            wm[64:128, c * 128 : (c + 1) * 128],
                            mt[64:128, :],
                            start=False, stop=True,
                        )
                    # 4) h1 eviction + PReLU (both chunks in one op via strided AP)
                    h1 = actp.tile([128, 800], bf16, tag="h1s")
                    nc.scalar.activation(
                        out=h1[:, :].rearrange("p (c x) -> p c x", c=2),
                        in_=ps_h1[:, :].rearrange("p (c x) -> p c x", c=2)[:, :, 0:400],
                        alpha=alphas[:, 0:1],
                        **prelu_kw,
                    )

                    # 5) GEMM2 -> ps_h2 ; eviction + PReLU
                    ps_h2 = psh2.tile([128, 400], mybir.dt.float32, tag="h2")
                    for c in range(2):
                        nc.tensor.matmul(
                            ps_h2[:, :], w2[:, c, :], h1[:, c * 400 : (c + 1) * 400],
                            start=(c == 0), stop=(c == 1),
                        )
                    h2 = actp.tile([128, 400], bf16, tag="h2s")
                    nc.scalar.activation(
                        out=h2[:, :], in_=ps_h2[:, :],
                        alpha=alphas[:, 1:2],
                        **prelu_kw,
                    )

                    # 6) GEMM3 -> ps_mix lower ; eviction + PReLU
                    ps_mix = psmix.tile([128, 400], mybir.dt.float32, tag="mix")
                    nc.tensor.matmul(
                        ps_mix[0:64, :], w3[:, :], h2[:, :], start=True, stop=True
                    )
                    h3 = actp.tile([64, 400], bf16, tag="h3s")
                    nc.scalar.activation(
                        out=h3[:, :], in_=ps_mix[0:64, :],
                        alpha=alphas[0:64, 2:3],
                        **prelu_kw,
                    )

                    # 7) replicated score -> ps_mix upper
                    nc.tensor.matmul(
                        ps_mix[64:128, :], wlr[:, :], h3[:, :],
                        start=True, stop=True, tile_position=(0, 64),
                    )

                    # 8) pooling: prod = kt * srep ; segmented reduce over t
                    prod = rhsp.tile([128, 400], f32, tag="prod")
                    nc.vector.tensor_tensor(
                        out=prod[64:128, :], in0=rhs[64:128, :], in1=ps_mix[64:128, :],
                        op=ALU.mult,
                    )
                    nc.vector.tensor_reduce(
                        out=outT[64:128, 2 * i : 2 * i + 2],
                        in_=prod[64:128, :].rearrange("p (s t) -> p s t", s=2),
                        op=ALU.add,
                        axis=mybir.AxisListType.X,
                    )

            nc.sync.dma_start(out=out_t[:, :], in_=outT[64:128, :])

    nc.finalize()
    return nc


def _get_nc():
    if "nc" not in _CACHE:
        _CACHE["nc"] = _build()
    return _CACHE["nc"]


def _prep_inputs(q, k, W1, W2, W3, Wl):
    """Host-side weight folding + per-core slicing. Returns in_maps list."""
    bf = ml_dtypes.bfloat16
    Wq = (W1[0:64] + W1[128:192]).astype(np.float32)
    Wk = (W1[64:128] - W1[128:192]).astype(np.float32)
    Wm = W1[192:256].astype(np.float32)
    wqk = np.concatenate([Wq, Wk], axis=0)                      # [128, 256]
    w2 = np.stack([W2[0:128], W2[128:256]])                     # [2, 128, 128]
    wlr = np.repeat(Wl.astype(np.float32), 64, axis=1)          # [64, 64]
    return wqk, Wm, w2, W3.astype(np.float32), wlr


def _run_jit(nc, in_maps):
    """Multi-core execute via cached jax jit (mirrors run_bass_via_pjrt)."""
    import jax
    import numpy as _np
    from jax.sharding import Mesh, PartitionSpec
    from jax.experimental.shard_map import shard_map
    from concourse import mybir
    from concourse import bass2jax
    from concourse.bass2jax import _bass_exec_p, partition_id_tensor

    key = "exec"
    if key not in _CACHE:
        bass2jax.install_neuronx_cc_hook()
        in_names = []
        out_names = []
        out_avals = []
        zero_outs = []
        partition_name = nc.partition_id_tensor.name if nc.partition_id_tensor else None
        for alloc in nc.m.functions[0].allocations:
            if not isinstance(alloc, mybir.MemoryLocationSet):
                continue
            name = alloc.memorylocations[0].name
            if alloc.kind == "ExternalInput":
                if name != partition_name:
                    in_names.append(name)
            elif alloc.kind == "ExternalOutput":
                shape = tuple(alloc.tensor_shape)
                dtype = mybir.dt.np(alloc.dtype)
                out_names.append(name)
                out_avals.append(jax.core.ShapedArray(shape, dtype))
                zero_outs.append(_np.zeros(shape, dtype))

        dbg_name = nc.dbg_addr.name if nc.dbg_addr is not None else None
        n_params = len(in_names)
        all_in_names = list(in_names) + out_names + ([partition_name] if partition_name else [])

        def _body(*args):
            operands = list(args)
            if partition_name is not None:
                operands.append(partition_id_tensor())
            outs = _bass_exec_p.bind(
                *operands,
                out_avals=tuple(out_avals),
                in_names=tuple(all_in_names),
                out_names=tuple(out_names),
                lowering_input_output_aliases=(),
                sim_require_finite=False,
                sim_require_nnan=False,
                nc=nc,
            )
            return tuple(outs)

        devices = jax.devices()[:M]
        mesh = Mesh(_np.asarray(devices), ("core",))
        donate = tuple(range(n_params, n_params + len(out_names)))
        sharded = jax.jit(
            shard_map(
                _body, mesh=mesh,
                in_specs=(PartitionSpec("core"),) * (n_params + len(out_names)),
                out_specs=(PartitionSpec("core"),) * len(out_names),
                check_rep=False,
            ),
            donate_argnums=donate,
            keep_unused=True,
        )
        _CACHE[key] = (sharded, in_names, out_names, out_avals, zero_outs, dbg_name)

    sharded, in_names, out_names, out_avals, zero_outs, dbg_name = _CACHE[key]
    per_core = [[_np.asarray(m[name]) for name in in_names] for m in in_maps]
    concat_in = [
        _np.concatenate([per_core[c][i] for c in range(M)], axis=0)
        for i in range(len(in_names))
    ]
    concat_zeros = [_np.zeros((M * z.shape[0], *z.shape[1:]), z.dtype) for z in zero_outs]
    out_arrs = sharded(*concat_in, *concat_zeros)
    return [
        {
            name: _np.asarray(out_arrs[i]).reshape(M, *out_avals[i].shape)[c]
            for i, name in enumerate(out_names)
        }
        for c in range(M)
    ]


def _make_in_maps(q, k, W1, W2, W3, Wl, a1, a2, a3):
    wqk, wm, w2, w3, wlr = _prep_inputs(q, k, W1, W2, W3, Wl)
    alphas = np.array([[a1.flat[0], a2.flat[0], a3.flat[0], 0.0]], dtype=np.float32)
    nc = _get_nc()
    dbg_zero = np.zeros((1, 2), np.uint32)
    in_maps = []
    for c in range(M):
        m = {
            "k": np.ascontiguousarray(k[c * BC : (c + 1) * BC]),
            "q": np.ascontiguousarray(q[c * BC : (c + 1) * BC, 0]),
            "wqk": wqk, "wm": wm, "w2": w2, "w3": w3, "wlr": wlr,
            "alphas": alphas,
        }
        if nc.dbg_addr is not None:
            m[nc.dbg_addr.name] = dbg_zero
        in_maps.append(m)
    return in_maps


def _numpy_fallback(q, k, W1, b1, a1, W2, b2, a2, W3, b3, a3, Wl, bl):
    def prelu(x, al):
        return np.maximum(x, 0) + al * np.minimum(x, 0)

    qt = np.broadcast_to(q, k.shape)
    att = np.concatenate([qt, k, qt - k, qt * k], axis=-1)
    h = prelu(att @ W1 + b1, a1)
    h = prelu(h @ W2 + b2, a2)
    h = prelu(h @ W3 + b3, a3)
    s = (h @ Wl + bl)[..., 0]
    s = np.where(k[:, :, 0] != 0, s, 0.0)
    return np.einsum("bt,btd->bd", s, k).astype(np.float32)


def kernel(q, k, W1, b1, a1, W2, b2, a2, W3, b3, a3, Wl, bl):
    q = np.asarray(q, np.float32); k = np.asarray(k, np.float32)
    W1 = np.asarray(W1, np.float32); W2 = np.asarray(W2, np.float32)
    W3 = np.asarray(W3, np.float32); Wl = np.asarray(Wl, np.float32)
    a1 = np.asarray(a1); a2 = np.asarray(a2); a3 = np.asarray(a3)
    b1 = np.asarray(b1); b2 = np.asarray(b2); b3 = np.asarray(b3)
    bl = np.asarray(bl)

    # conditions the fast path relies on; fall back to exact numpy otherwise
    special = (
        np.ptp(a1) == 0 and np.ptp(a2) == 0 and np.ptp(a3) == 0
        and not np.any(b1) and not np.any(b2) and not np.any(b3) and not np.any(bl)
        and not np.any(k[:, :, 0] == 0.0)
    )
    if not special:
        return _numpy_fallback(q, k, W1, b1, a1, W2, b2, a2, W3, b3, a3, Wl, bl)

    in_maps = _make_in_maps(q, k, W1, W2, W3, Wl, a1, a2, a3)
    results = _run_jit(_get_nc(), in_maps)
    out = np.empty((B, D), np.float32)
    for c in range(M):
        out[c * BC : (c + 1) * BC] = results[c]["out_t"].T
    return out


def profile(q, k, W1, b1, a1, W2, b2, a2, W3, b3, a3, Wl, bl):
    """Run once via run_bass_kernel_spmd(trace=True); returns (out, exec_time_ns)."""
    from concourse.bass_utils import run_bass_kernel_spmd

    q = np.asarray(q, np.float32); k = np.asarray(k, np.float32)
    in_maps = _make_in_maps(
        q, k, np.asarray(W1, np.float32), np.asarray(W2, np.float32),
        np.asarray(W3, np.float32), np.asarray(Wl, np.float32),
        np.asarray(a1), np.asarray(a2), np.asarray(a3),
    )
    # run_bass_kernel_spmd supplies dbg/partition tensors itself
    nc = _get_nc()
    if nc.dbg_addr is not None:
        for m in in_maps:
            m.pop(nc.dbg_addr.name, None)
    res = run_bass_kernel_spmd(nc, in_maps, core_ids=list(range(M)), trace=True)
    out = np.empty((B, D), np.float32)
    for c in range(M):
        out[c * BC : (c + 1) * BC] = res.results[c]["out_t"].T
    return out, res
